# revision 1
# baseline (speedup 1.0000x reference)
"""GatedConv GNN message passing on 8 TRN2 NeuronCores.

Strategy:
- Nodes sharded contiguously across 8 cores (6250/core, padded to 6272=49*128).
- Edges sharded by dst owner, sorted by dst, grouped into 128-node dst blocks,
  padded to a uniform tiles-per-block capacity so one SPMD program serves all
  cores.
- Per layer: AllGather h (bf16) -> per 128-edge tile: indirect-DMA row gather
  of h_full[src] + host-precomputed one-hot dst mask -> PE matmul
  (h_g.T @ mask) accumulated in PSUM per dst block = transposed segment sum.
  Conv weight is folded AFTER aggregation (linearity). GRU runs in transposed
  [feature, node] layout; PE transposes produce the row-major h for the next
  AllGather / final pooling.
- Mean-pool via host-built batch one-hot matmul + 1/count scale; host sums the
  8 per-core partials (unshard-reduce).
"""
import contextlib
import ctypes
import os
import sys
import types

import numpy as np

from concourse import bass, mybir, tile
from concourse.bass_utils import run_bass_kernel_spmd

NCORES = 8
P = 128
D = 128
G = 64
N = 50000
V = 100000
NUM_LAYERS = 2
NL = N // NCORES            # 6250 nodes per core
NB = (NL + P - 1) // P      # 49 dst blocks per core
NLP = NB * P                # 6272 padded nodes per core
NFULL = NCORES * NLP        # 50176 rows in allgathered h

_F32 = mybir.dt.float32
_BF16 = mybir.dt.bfloat16
_I32 = mybir.dt.int32


# ---------------------------------------------------------------- wait split
def _split_waits(nc):
    """walrus allows only ONE sync-wait per instruction; hoist extras onto
    NoOps just before, on the same engine stream (sequencer order)."""
    uid = 0
    n_fixed = 0
    for bb in nc.main_func.blocks:
        out = []
        for ins in bb.instructions:
            si = getattr(ins, "sync_info", None)
            if si is not None and len(si.on_wait) > 1:
                for w in si.on_wait[:-1]:
                    uid += 1
                    out.append(mybir.InstNoOp(
                        name=f"WSPLIT-{uid}", engine=ins.engine,
                        bass_nofuse=True, ins=[], outs=[],
                        sync_info=mybir.SyncInfo(on_wait=[w], on_update=[]),
                    ))
                ins.sync_info = mybir.SyncInfo(
                    on_wait=[si.on_wait[-1]], on_update=si.on_update)
                n_fixed += 1
            out.append(ins)
        bb.instructions = out
    return n_fixed


# ---------------------------------------------------------------- ntff hook
def _install_ntff_hook():
    import antenv
    if "antenv.axon_hooks" in sys.modules:
        return
    mod = types.ModuleType("antenv.axon_hooks")
    _state = {"hook": None}
    mod.set_axon_ntff_profile_hook = lambda h: _state.__setitem__("hook", h)
    mod.get_axon_ntff_profile_hook = lambda: _state["hook"]
    sys.modules["antenv.axon_hooks"] = mod
    antenv.axon_hooks = mod
    if "/root/.axon_site" not in sys.path:
        sys.path.insert(0, "/root/.axon_site")
    try:
        from trn_agent_boot.trn_boot import _ntff_profile_via_ctypes
        hook = _ntff_profile_via_ctypes("/opt/axon/libaxon_pjrt.so")
        mod.set_axon_ntff_profile_hook(hook)
    except Exception:
        pass


# ---------------------------------------------------------------- builder
def _build(cap: int, phases: int = 99):
    """cap = max edge tiles per (core, dst-block); uniform across cores."""
    nc = bass.Bass(num_devices=NCORES)
    T = NB * cap  # edge tiles per core per layer

    embed_in = nc.declare_dram_parameter("embed", [V, D], _F32, isOutput=False)
    nid_in = nc.declare_dram_parameter("nid", [P, NB], _I32, isOutput=False)
    src_in = nc.declare_dram_parameter("srcidx", [P, T], _I32, isOutput=False)
    mask_in = nc.declare_dram_parameter("masks", [T * P, D], _BF16, isOutput=False)
    pool_in = nc.declare_dram_parameter("pool1h", [P, NB * G], _BF16, isOutput=False)
    cinv_in = nc.declare_dram_parameter("cinv", [G, 1], _F32, isOutput=False)
    convw_in = nc.declare_dram_parameter("convw", [D, NUM_LAYERS * D], _F32, isOutput=False)
    wih_in = nc.declare_dram_parameter("wihT", [D, 3 * D], _F32, isOutput=False)
    whh_in = nc.declare_dram_parameter("whhT", [D, 3 * D], _F32, isOutput=False)
    bias_in = nc.declare_dram_parameter("biases", [P, 4], _F32, isOutput=False)
    out_ext = nc.declare_dram_parameter("out", [G, D], _F32, isOutput=True)

    ag_in = [nc.dram_tensor(f"ag_in{l}", [NLP, D], _BF16) for l in range(NUM_LAYERS)]
    ag_out = [nc.dram_tensor(f"ag_out{l}", [NFULL, D], _BF16, addr_space="Shared")
              for l in range(NUM_LAYERS)]

    with tile.TileContext(nc) as tc:
        with contextlib.ExitStack() as stk:
            const = stk.enter_context(tc.tile_pool(name="const", bufs=1))
            sb = stk.enter_context(tc.tile_pool(name="sb", bufs=3))
            pp = stk.enter_context(tc.tile_pool(name="pp", bufs=2, space="PSUM"))
            gpsum = stk.enter_context(tc.tile_pool(name="gpsum", bufs=1, space="PSUM"))

            # ---- constants / weights ----
            src_sb = const.tile([P, T], _I32)
            nc.sync.dma_start(out=src_sb[:], in_=src_in[:])
            nid_sb = const.tile([P, NB], _I32)
            nc.sync.dma_start(out=nid_sb[:], in_=nid_in[:])
            pool_sb = const.tile([P, NB * G], _BF16)
            nc.sync.dma_start(out=pool_sb[:], in_=pool_in[:])
            cinv_sb = const.tile([G, 1], _F32)
            nc.sync.dma_start(out=cinv_sb[:], in_=cinv_in[:])
            bias_sb = const.tile([P, 4], _F32)
            nc.sync.dma_start(out=bias_sb[:], in_=bias_in[:])

            def _load_bf16(src_ap, shape, nm):
                t32 = sb.tile(shape, _F32, name=f"t32_{nm}", tag=f"t32_{nm}")
                nc.sync.dma_start(out=t32[:], in_=src_ap)
                tb = const.tile(shape, _BF16, name=f"bf_{nm}", tag=f"bf_{nm}")
                nc.scalar.copy(out=tb[:], in_=t32[:])
                return tb

            convw_sb = _load_bf16(convw_in[:], [D, NUM_LAYERS * D], "convw")
            wih_sb = _load_bf16(wih_in[:], [D, 3 * D], "wih")
            whh_sb = _load_bf16(whh_in[:], [D, 3 * D], "whh")

            from concourse.masks import make_identity
            ident = const.tile([P, P], _BF16)
            make_identity(nc, ident[:])

            # ---- persistent state buffers ----
            hT = [const.tile([P, NLP], _BF16, name=f"hT{i}", tag=f"hT{i}") for i in range(2)]
            hnorm = const.tile([P, NLP], _BF16)   # [node-part, d] per 128-block, col-block b
            aggT = const.tile([P, NLP], _BF16)

            # ---- phase 1: embed gather -> hnorm + hT[0] ----
            for b in range(NB):
                g32 = sb.tile([P, D], _F32, tag="embg")
                nc.gpsimd.indirect_dma_start(
                    out=g32[:], out_offset=None, in_=embed_in[:],
                    in_offset=bass.IndirectOffsetOnAxis(ap=nid_sb[:, b:b + 1], axis=0))
                nc.scalar.copy(out=hnorm[:, b * D:(b + 1) * D], in_=g32[:])
                tp = pp.tile([P, P], _BF16, tag="scratch", space="PSUM")
                nc.tensor.transpose(out=tp[:], in_=hnorm[:, b * D:(b + 1) * D], identity=ident[:])
                nc.scalar.copy(out=hT[0][:, b * P:(b + 1) * P], in_=tp[:])
            nc.sync.dma_start(
                out=ag_in[0][:].rearrange("(b p) d -> p b d", p=P),
                in_=hnorm[:].rearrange("p (b d) -> p b d", d=D))

            # ---- layers ----
            for l in range(NUM_LAYERS if phases >= 2 else 0):
                nc.gpsimd.collective_compute(
                    "AllGather", mybir.AluOpType.bypass,
                    replica_groups=[list(range(NCORES))],
                    ins=[ag_in[l][:]], outs=[ag_out[l][:]])

                # edge phase: per dst block, segment-sum via mask matmuls in PSUM
                for b in range(NB):
                    pagg = pp.tile([P, P], _F32, tag="scratch", space="PSUM")
                    mblk = sb.tile([P, cap * D], _BF16, tag="mblk")
                    nc.sync.dma_start(
                        out=mblk[:].rearrange("p (t d) -> p t d", d=D),
                        in_=mask_in[b * cap * P:(b + 1) * cap * P, :].rearrange(
                            "(t p) d -> p t d", p=P))
                    for t in range(cap):
                        tt = b * cap + t
                        gt = sb.tile([P, D], _BF16, tag="gath")
                        nc.gpsimd.indirect_dma_start(
                            out=gt[:], out_offset=None, in_=ag_out[l][:],
                            in_offset=bass.IndirectOffsetOnAxis(ap=src_sb[:, tt:tt + 1], axis=0))
                        nc.tensor.matmul(out=pagg[:], lhsT=gt[:], rhs=mblk[:, t * D:(t + 1) * D],
                                         start=(t == 0), stop=(t == cap - 1))
                    nc.scalar.copy(out=aggT[:, b * P:(b + 1) * P], in_=pagg[:])

                if phases < 3:
                    continue
                # conv + GRU phase, slabs of 512 nodes
                W = 512
                nslab = NLP // W if NLP % W == 0 else NLP // W + 1
                hT_next = hT[(l + 1) % 2]
                for s in range(nslab):
                    c0 = s * W
                    w = min(W, NLP - c0)
                    cs = slice(c0, c0 + w)
                    xt_ps = gpsum.tile([P, W], _F32, tag="gi0", space="PSUM")
                    nc.tensor.matmul(out=xt_ps[:, :w], lhsT=convw_sb[:, l * D:(l + 1) * D],
                                     rhs=aggT[:, cs], start=True, stop=True)
                    xt_sb = sb.tile([P, W], _BF16, tag="xtsb")
                    nc.scalar.copy(out=xt_sb[:, :w], in_=xt_ps[:, :w])

                    gi = []
                    gh = []
                    for gidx in range(3):
                        gps = gpsum.tile([P, W], _F32, tag=f"gi{gidx}", space="PSUM")
                        nc.tensor.matmul(out=gps[:, :w], lhsT=wih_sb[:, gidx * D:(gidx + 1) * D],
                                         rhs=xt_sb[:, :w], start=True, stop=True)
                        gi.append(gps)
                        hps = gpsum.tile([P, W], _F32, tag=f"gh{gidx}", space="PSUM")
                        nc.tensor.matmul(out=hps[:, :w], lhsT=whh_sb[:, gidx * D:(gidx + 1) * D],
                                         rhs=hT[l % 2][:, cs], start=True, stop=True)
                        gh.append(hps)

                    # r = sigmoid(gi_r + gh_r + b_r) ; z likewise
                    r_sb = sb.tile([P, W], _F32, tag="r")
                    nc.scalar.activation(out=r_sb[:, :w], in_=gh[0][:, :w],
                                         func=mybir.ActivationFunctionType.Identity,
                                         bias=bias_sb[:, 0:1])
                    nc.vector.tensor_tensor(out=r_sb[:, :w], in0=gi[0][:, :w], in1=r_sb[:, :w],
                                            op=mybir.AluOpType.add)
                    nc.scalar.activation(out=r_sb[:, :w], in_=r_sb[:, :w],
                                         func=mybir.ActivationFunctionType.Sigmoid)
                    z_sb = sb.tile([P, W], _F32, tag="z")
                    nc.scalar.activation(out=z_sb[:, :w], in_=gh[1][:, :w],
                                         func=mybir.ActivationFunctionType.Identity,
                                         bias=bias_sb[:, 1:2])
                    nc.vector.tensor_tensor(out=z_sb[:, :w], in0=gi[1][:, :w], in1=z_sb[:, :w],
                                            op=mybir.AluOpType.add)
                    nc.scalar.activation(out=z_sb[:, :w], in_=z_sb[:, :w],
                                         func=mybir.ActivationFunctionType.Sigmoid)
                    # n = tanh(gi_n + b_in + r * (gh_n + b_hn))
                    hn_sb = sb.tile([P, W], _F32, tag="hn")
                    nc.scalar.activation(out=hn_sb[:, :w], in_=gh[2][:, :w],
                                         func=mybir.ActivationFunctionType.Identity,
                                         bias=bias_sb[:, 3:4])
                    nc.vector.tensor_tensor(out=hn_sb[:, :w], in0=r_sb[:, :w], in1=hn_sb[:, :w],
                                            op=mybir.AluOpType.mult)
                    nc.vector.tensor_tensor(out=hn_sb[:, :w], in0=hn_sb[:, :w], in1=gi[2][:, :w],
                                            op=mybir.AluOpType.add)
                    nc.scalar.activation(out=hn_sb[:, :w], in_=hn_sb[:, :w],
                                         func=mybir.ActivationFunctionType.Tanh,
                                         bias=bias_sb[:, 2:3])
                    # h' = n + z*(h - n)
                    d_sb = sb.tile([P, W], _F32, tag="d")
                    nc.vector.tensor_tensor(out=d_sb[:, :w], in0=hT[l % 2][:, cs], in1=hn_sb[:, :w],
                                            op=mybir.AluOpType.subtract)
                    nc.vector.tensor_tensor(out=d_sb[:, :w], in0=z_sb[:, :w], in1=d_sb[:, :w],
                                            op=mybir.AluOpType.mult)
                    nc.vector.tensor_tensor(out=hT_next[:, cs], in0=d_sb[:, :w], in1=hn_sb[:, :w],
                                            op=mybir.AluOpType.add)

                # transpose h'T back to row-major hnorm
                for b in range(NB):
                    tp = pp.tile([P, P], _BF16, tag="scratch", space="PSUM")
                    nc.tensor.transpose(out=tp[:], in_=hT_next[:, b * P:(b + 1) * P],
                                        identity=ident[:])
                    nc.scalar.copy(out=hnorm[:, b * D:(b + 1) * D], in_=tp[:])
                if l + 1 < NUM_LAYERS:
                    nc.sync.dma_start(
                        out=ag_in[l + 1][:].rearrange("(b p) d -> p b d", p=P),
                        in_=hnorm[:].rearrange("p (b d) -> p b d", d=D))

            # ---- pool ----
            if phases < 4:
                out_sb0 = sb.tile([G, D], _F32, tag="outsb")
                nc.vector.memset(out_sb0[:], 0.0)
                nc.sync.dma_start(out=out_ext[:], in_=out_sb0[:])
            else:
                ppool = pp.tile([G, D], _F32, tag="scratch", space="PSUM")
                for b in range(NB):
                    nc.tensor.matmul(out=ppool[:], lhsT=pool_sb[:, b * G:(b + 1) * G],
                                     rhs=hnorm[:, b * D:(b + 1) * D],
                                     start=(b == 0), stop=(b == NB - 1))
                out_sb = sb.tile([G, D], _F32, tag="outsb")
                nc.vector.tensor_scalar(out=out_sb[:], in0=ppool[:], scalar1=cinv_sb[:, 0:1],
                                        scalar2=None, op0=mybir.AluOpType.mult)
                nc.sync.dma_start(out=out_ext[:], in_=out_sb[:])

    _split_waits(nc)
    return nc


_CACHE = {}


def kernel(node_ids, edge_index, batch, num_graphs, embed, conv_w, w_ih, w_hh,
           b_ih, b_hh) -> np.ndarray:
    import ml_dtypes
    bf16 = ml_dtypes.bfloat16

    node_ids = np.asarray(node_ids)
    edge_index = np.asarray(edge_index)
    batch = np.asarray(batch)
    embed = np.asarray(embed, dtype=np.float32)
    conv_w = np.asarray(conv_w, dtype=np.float32)
    w_ih = np.asarray(w_ih, dtype=np.float32)
    w_hh = np.asarray(w_hh, dtype=np.float32)
    b_ih = np.asarray(b_ih, dtype=np.float32)
    b_hh = np.asarray(b_hh, dtype=np.float32)
    G_ = int(num_graphs)
    assert G_ == G and node_ids.shape[0] == N

    src_all = edge_index[0].astype(np.int64)
    dst_all = edge_index[1].astype(np.int64)

    # shard edges by dst owner; per (core, block) group edges; uniform capacity
    owner = dst_all // NL
    per_core = []
    max_tiles = 1
    for c in range(NCORES):
        sel = owner == c
        src_c = src_all[sel]
        dst_c = dst_all[sel] - c * NL          # 0..NL-1
        blk = dst_c // P
        rel = dst_c % P
        order = np.argsort(blk * P + rel, kind="stable")
        src_c, blk, rel = src_c[order], blk[order], rel[order]
        counts = np.bincount(blk, minlength=NB)
        max_tiles = max(max_tiles, int(np.ceil(counts.max() / P)))
        per_core.append((src_c, blk, rel, counts))
    cap = max_tiles
    T = NB * cap

    # global padded row index of node n in ag_out
    def padded_idx(n):
        return (n // NL) * NLP + (n % NL)

    in_maps = []
    # common tensors
    convw_arr = np.ascontiguousarray(np.concatenate([conv_w[i] for i in range(NUM_LAYERS)], axis=1))
    wihT = np.ascontiguousarray(w_ih.T)           # [128, 384]
    whhT = np.ascontiguousarray(w_hh.T)
    biases = np.zeros((P, 4), np.float32)
    biases[:, 0] = b_ih[0:D] + b_hh[0:D]          # r
    biases[:, 1] = b_ih[D:2 * D] + b_hh[D:2 * D]  # z
    biases[:, 2] = b_ih[2 * D:3 * D]              # in
    biases[:, 3] = b_hh[2 * D:3 * D]              # hn
    counts_g = np.bincount(batch, minlength=G).astype(np.float32)
    cinv = (1.0 / np.maximum(counts_g, 1.0)).reshape(G, 1).astype(np.float32)

    eye = np.eye(P, dtype=bf16)
    for c in range(NCORES):
        src_c, blk, rel, counts = per_core[c]
        srcidx = np.zeros((P, T), np.int32)
        masks = np.zeros((T * P, D), dtype=bf16)
        pos = 0
        for b in range(NB):
            nb_e = int(counts[b])
            e_src = padded_idx(src_c[pos:pos + nb_e]).astype(np.int32)
            e_rel = rel[pos:pos + nb_e].astype(np.int64)
            pos += nb_e
            for t in range(cap):
                tt = b * cap + t
                lo = t * P
                sl_src = e_src[lo:lo + P]
                sl_rel = e_rel[lo:lo + P]
                k = sl_src.shape[0]
                if k:
                    srcidx[:k, tt] = sl_src
                    masks[tt * P:tt * P + k, :] = eye[sl_rel]
        # node ids per padded slot, [128, NB] column-major tiles
        nid = np.zeros((P, NB), np.int32)
        ids_c = node_ids[c * NL:(c + 1) * NL].astype(np.int32)
        ids_pad = np.zeros(NLP, np.int32)
        ids_pad[:NL] = ids_c
        nid[:, :] = ids_pad.reshape(NB, P).T
        # pool one-hot [128, NB*G]
        b_c = batch[c * NL:(c + 1) * NL].astype(np.int64)
        p1h = np.zeros((NLP, G), dtype=bf16)
        p1h[np.arange(NL), b_c] = np.float32(1.0)
        pool1h = np.zeros((P, NB * G), dtype=bf16)
        for b in range(NB):
            pool1h[:, b * G:(b + 1) * G] = p1h[b * P:(b + 1) * P, :]

        in_maps.append({
            "embed": embed, "nid": nid, "srcidx": srcidx, "masks": masks,
            "pool1h": pool1h, "cinv": cinv, "convw": convw_arr,
            "wihT": wihT, "whhT": whhT, "biases": biases,
        })

    if cap not in _CACHE:
        _CACHE[cap] = _build(cap)
    nc = _CACHE[cap]

    trace = bool(int(os.environ.get("BASS_GNN_TRACE", "0")))
    if trace:
        _install_ntff_hook()
    res = run_bass_kernel_spmd(nc, in_maps, core_ids=list(range(NCORES)),
                               trace=trace)
    if trace:
        kernel.last_exec_time_ns = res.exec_time_ns
        kernel.last_results = res
    outs = [r["out"] for r in res.results]
    return np.sum(np.stack(outs, 0), axis=0, dtype=np.float32)


kernel.last_exec_time_ns = None



# revision 14
# speedup vs baseline: 1.8702x; 1.8702x over previous
"""GatedConv GNN message passing on 8 TRN2 NeuronCores — v7.

Bottleneck model (HW-measured): per-row gathers cost ~8.2ns/row of Q7
SWDGE descriptor generation per queue; 4 SWDGE queues run concurrently
(~2.4ns/row effective), so the gather pipeline dominates. Design:

- All row gathers use batched InstDMAGatherAnt (gpsimd.dma_gather, mlp
  ucode library), round-robin over 4 SWDGE queues, uniform 28-tile
  (3584-row) chunks sharing one num_idxs register.
- Layer 0 gathers per-edge h directly from the (bf16) embedding table —
  no AllGather needed for layer 0, and its gathers start at t=0. Only
  layer 1 needs an AllGather of the GRU output.
- dma_gather idxs are int16, so each gather series reads a <=32768-row
  table slice: 4 embed-id ranges for layer 0, lo/hi halves of the padded
  global node space for layer 1.
- Edges sharded by dst owner, grouped into 128-slot tiles per
  (dst-block, range-group); within a group, edges sharing a source row
  are DEDUPED into one gathered slot (the one-hot dst mask rows become
  multi-hot/counted). Per-(block,group) tile capacity uniform across
  cores.
- Aggregation: per tile, PE matmul (gathered.T @ mask) accumulated in
  PSUM per dst block = transposed segment sum. Conv weight folded after
  aggregation (linearity). Masks are host-built, stored [128, T*128] so
  each per-block load is one contiguous HWDGE DMA on the Activation
  queue.
- GRU runs in transposed [feature, node] layout; PE transposes produce
  row-major h only where needed (pooling, AllGather input).
- Mean-pool via host-built batch one-hot matmul + 1/count scale; host
  sums the 8 per-core partials.
"""
import contextlib
import os
import sys
import types

import numpy as np

from concourse import bass, mybir, tile, library_config
from concourse.bass_utils import run_bass_kernel_spmd
from concourse.library_overlay import lower_extended_insts

NCORES = 8
P = 128
D = 128
G = 64
N = 50000
V = 100000
NUM_LAYERS = 2
NL = N // NCORES            # 6250 nodes per core
NRANGE = 4                  # embed id ranges (V/NRANGE < 32768)
VR = V // NRANGE            # 25000
GMAX = 28                   # tiles per dma_gather (3584 idxs)

_F32 = mybir.dt.float32
_BF16 = mybir.dt.bfloat16
_I16 = mybir.dt.int16


def _split_waits(nc):
    """walrus allows only ONE sync-wait per instruction; hoist extras onto
    NoOps just before, on the same engine stream (sequencer order)."""
    uid = 0
    for bb in nc.main_func.blocks:
        out = []
        for ins in bb.instructions:
            si = getattr(ins, "sync_info", None)
            if si is not None and len(si.on_wait) > 1:
                for w in si.on_wait[:-1]:
                    uid += 1
                    out.append(mybir.InstNoOp(
                        name=f"WSPLIT-{uid}", engine=ins.engine,
                        bass_nofuse=True, ins=[], outs=[],
                        sync_info=mybir.SyncInfo(on_wait=[w], on_update=[]),
                    ))
                ins.sync_info = mybir.SyncInfo(
                    on_wait=[si.on_wait[-1]], on_update=si.on_update)
            out.append(ins)
        bb.instructions = out


def _install_ntff_hook():
    import antenv
    if "antenv.axon_hooks" in sys.modules:
        return
    mod = types.ModuleType("antenv.axon_hooks")
    _state = {"hook": None}
    mod.set_axon_ntff_profile_hook = lambda h: _state.__setitem__("hook", h)
    mod.get_axon_ntff_profile_hook = lambda: _state["hook"]
    sys.modules["antenv.axon_hooks"] = mod
    antenv.axon_hooks = mod
    if "/root/.axon_site" not in sys.path:
        sys.path.insert(0, "/root/.axon_site")
    try:
        from trn_agent_boot.trn_boot import _ntff_profile_via_ctypes
        hook = _ntff_profile_via_ctypes("/opt/axon/libaxon_pjrt.so")
        mod.set_axon_ntff_profile_hook(hook)
    except Exception:
        pass


# ---------------------------------------------------------------- builder
def _build(sig):
    """sig = (NB, CAPR, caps0, caps1); capsL = flattened [NB][R_l] tile caps."""
    NB, CAPR, caps0_f, caps1_f = sig
    NB = int(NB)
    CAPR = list(CAPR)
    R = [NRANGE, 2]
    caps = [np.array(caps0_f, int).reshape(NB, R[0]),
            np.array(caps1_f, int).reshape(NB, R[1])]
    NLP = NB * P
    NFULL = NCORES * NLP
    HALF = NFULL // 2
    T_EMB = sum(CAPR)
    EmbOff = np.concatenate([[0], np.cumsum(CAPR)[:-1]]).astype(int)

    SOff = []       # SOff[l][r][b] : tile offset of block b in series r
    TileOff = []    # TileOff[l][b] : mask-order tile offset of block b
    NG = []         # NG[l][r] : number of GMAX windows in series r
    CAPB = []       # CAPB[l][b] : total tiles of block b
    GrpOff = []     # GrpOff[l][b][r] : within-block tile offset of group r
    T_TOT = []
    for l in range(NUM_LAYERS):
        cl = caps[l]
        SOff.append([np.concatenate([[0], np.cumsum(cl[:, r])[:-1]]).astype(int)
                     for r in range(R[l])])
        capb = cl.sum(axis=1)
        CAPB.append(capb.astype(int))
        TileOff.append(np.concatenate([[0], np.cumsum(capb)[:-1]]).astype(int))
        NG.append([(int(cl[:, r].sum()) + GMAX - 1) // GMAX for r in range(R[l])])
        GrpOff.append(np.concatenate(
            [np.zeros((NB, 1), int), np.cumsum(cl, axis=1)[:, :-1]], axis=1))
        T_TOT.append(int(capb.sum()))
    CAPMAX = int(max(CAPB[0].max(), CAPB[1].max()))

    nc = bass.Bass(num_devices=NCORES, num_swdge_queues=4)

    embed_in = nc.declare_dram_parameter("embed", [V, D], _BF16, isOutput=False)
    idxe_in = nc.declare_dram_parameter("idxemb", [P, T_EMB * 8], _I16, isOutput=False)
    idx_in = [[nc.declare_dram_parameter(f"idx{l}_{r}", [P, NG[l][r] * GMAX * 8],
                                         _I16, isOutput=False)
               for r in range(R[l])] for l in range(NUM_LAYERS)]
    mask_in = [nc.declare_dram_parameter(f"masks{l}", [P, T_TOT[l] * P], _BF16,
                                         isOutput=False)
               for l in range(NUM_LAYERS)]
    ident_in = nc.declare_dram_parameter("ident", [P, P], _BF16, isOutput=False)
    pool_in = nc.declare_dram_parameter("pool1h", [P, NB * G], _BF16, isOutput=False)
    cinv_in = nc.declare_dram_parameter("cinv", [G, 1], _F32, isOutput=False)
    convw_in = nc.declare_dram_parameter("convw", [D, NUM_LAYERS * D], _BF16, isOutput=False)
    wih_in = nc.declare_dram_parameter("wihT", [D, 3 * D], _BF16, isOutput=False)
    whh_in = nc.declare_dram_parameter("whhT", [D, 3 * D], _BF16, isOutput=False)
    bias_in = nc.declare_dram_parameter("biases", [P, 4], _F32, isOutput=False)
    out_ext = nc.declare_dram_parameter("out", [G, D], _F32, isOutput=True)

    ag_in = nc.dram_tensor("ag_in", [NLP, D], _BF16)
    ag_out = nc.dram_tensor("ag_out", [NFULL, D], _BF16, addr_space="Shared")

    with tile.TileContext(nc) as tc:
        with contextlib.ExitStack() as stk:
            const = stk.enter_context(tc.tile_pool(name="const", bufs=1))
            sb = stk.enter_context(tc.tile_pool(name="sb", bufs=3))
            gpool = stk.enter_context(tc.tile_pool(name="gpool", bufs=2))
            mpool = stk.enter_context(tc.tile_pool(name="mpool", bufs=3))
            pp = stk.enter_context(tc.tile_pool(name="pp", bufs=2, space="PSUM"))
            gpsum = stk.enter_context(tc.tile_pool(name="gpsum", bufs=1, space="PSUM"))

            # ---- constants ----
            idxe_sb = const.tile([P, T_EMB * 8], _I16)
            nc.sync.dma_start(out=idxe_sb[:], in_=idxe_in[:])
            idx_sb = [[const.tile([P, NG[l][r] * GMAX * 8], _I16,
                                  name=f"idx{l}{r}", tag=f"idx{l}{r}")
                       for r in range(R[l])] for l in range(NUM_LAYERS)]
            for l in range(NUM_LAYERS):
                for r in range(R[l]):
                    nc.sync.dma_start(out=idx_sb[l][r][:], in_=idx_in[l][r][:])
            ident = const.tile([P, P], _BF16)
            nc.sync.dma_start(out=ident[:], in_=ident_in[:])
            pool_sb = const.tile([P, NB * G], _BF16)
            nc.sync.dma_start(out=pool_sb[:], in_=pool_in[:])
            cinv_sb = const.tile([G, 1], _F32)
            nc.sync.dma_start(out=cinv_sb[:], in_=cinv_in[:])
            bias_sb = const.tile([P, 4], _F32)
            nc.sync.dma_start(out=bias_sb[:], in_=bias_in[:])
            convw_sb = const.tile([D, NUM_LAYERS * D], _BF16)
            nc.sync.dma_start(out=convw_sb[:], in_=convw_in[:])
            wih_sb = const.tile([D, 3 * D], _BF16)
            nc.sync.dma_start(out=wih_sb[:], in_=wih_in[:])
            whh_sb = const.tile([D, 3 * D], _BF16)
            nc.sync.dma_start(out=whh_sb[:], in_=whh_in[:])

            # gpsimd carries only the mlp ucode library + dma_gathers
            nc.gpsimd.load_library(library_config.mlp)
            gnreg = nc.gpsimd.to_reg(GMAX * P)
            enregs = {}
            for r in range(NRANGE):
                if CAPR[r] and CAPR[r] * P not in enregs:
                    enregs[CAPR[r] * P] = nc.gpsimd.to_reg(CAPR[r] * P)

            hT = [const.tile([P, NLP], _BF16, name=f"hT{i}", tag=f"hT{i}")
                  for i in range(2)]
            hnorm = const.tile([P, NLP], _BF16)
            aggT = const.tile([P, NLP], _BF16)

            qctr = [0]

            def edge_layer(l, table_slices):
                """table_slices[r] = DRAM AP of the gather table for series r."""
                bufs = {}
                cl = caps[l]

                def issue(r, g):
                    # tags shared across layers (phases are sequential) to
                    # halve the static gpool footprint
                    buf = gpool.tile([P, GMAX * D], _BF16, tag=f"g{r}")
                    bufs[(r, g)] = buf
                    nc.gpsimd.dma_gather(
                        out_ap=buf[:].rearrange("p (t d) -> p t d", d=D),
                        in_ap=table_slices[r],
                        idxs_ap=idx_sb[l][r][:, g * GMAX * 8:(g + 1) * GMAX * 8],
                        num_idxs=GMAX * P, num_idxs_reg=gnreg,
                        elem_size=D, single_packet=False,
                        queue_num=qctr[0] % 4)
                    qctr[0] += 1

                def blk_need(b):
                    need = 0
                    for r in range(R[l]):
                        if cl[b][r]:
                            need = max(need,
                                       (int(SOff[l][r][b]) + cl[b][r] - 1) // GMAX)
                    return need

                n_g = max(NG[l])
                next_b = 0
                for g in range(n_g):
                    for r in range(R[l]):
                        if g < NG[l][r]:
                            issue(r, g)
                    while next_b < NB and blk_need(next_b) <= g:
                        b = next_b
                        next_b += 1
                        capb = int(CAPB[l][b])
                        if capb == 0:
                            nc.vector.memset(aggT[:, b * P:(b + 1) * P], 0.0)
                            continue
                        mask = mpool.tile([P, CAPMAX * P], _BF16, tag="mask")
                        to = int(TileOff[l][b])
                        nc.scalar.dma_start(
                            out=mask[:, :capb * P],
                            in_=mask_in[l][:, to * P:(to + capb) * P])
                        pagg = pp.tile([P, P], _F32, tag="scratch", space="PSUM")
                        k = 0
                        for r in range(R[l]):
                            for t in range(cl[b][r]):
                                st = int(SOff[l][r][b]) + t
                                buf = bufs[(r, st // GMAX)]
                                col = st % GMAX
                                nc.tensor.matmul(
                                    out=pagg[:],
                                    lhsT=buf[:, col * D:(col + 1) * D],
                                    rhs=mask[:, k * P:(k + 1) * P],
                                    start=(k == 0), stop=(k == capb - 1))
                                k += 1
                        nc.scalar.copy(out=aggT[:, b * P:(b + 1) * P], in_=pagg[:])
                assert next_b == NB

            def gru_layer(l):
                W = 512
                nslab = (NLP + W - 1) // W
                hT_next = hT[(l + 1) % 2]
                for s in range(nslab):
                    c0 = s * W
                    w = min(W, NLP - c0)
                    cs = slice(c0, c0 + w)
                    xt_ps = gpsum.tile([P, W], _F32, tag="gi0", space="PSUM")
                    nc.tensor.matmul(out=xt_ps[:, :w],
                                     lhsT=convw_sb[:, l * D:(l + 1) * D],
                                     rhs=aggT[:, cs], start=True, stop=True)
                    xt_sb = sb.tile([P, W], _BF16, tag="xtsb")
                    nc.scalar.copy(out=xt_sb[:, :w], in_=xt_ps[:, :w])

                    gi = []
                    gh = []
                    for gidx in range(3):
                        gps = gpsum.tile([P, W], _F32, tag=f"gi{gidx}", space="PSUM")
                        nc.tensor.matmul(out=gps[:, :w],
                                         lhsT=wih_sb[:, gidx * D:(gidx + 1) * D],
                                         rhs=xt_sb[:, :w], start=True, stop=True)
                        gi.append(gps)
                        hps = gpsum.tile([P, W], _F32, tag=f"gh{gidx}", space="PSUM")
                        nc.tensor.matmul(out=hps[:, :w],
                                         lhsT=whh_sb[:, gidx * D:(gidx + 1) * D],
                                         rhs=hT[l % 2][:, cs], start=True, stop=True)
                        gh.append(hps)

                    r_sb = sb.tile([P, W], _F32, tag="r")
                    nc.scalar.activation(out=r_sb[:, :w], in_=gh[0][:, :w],
                                         func=mybir.ActivationFunctionType.Identity,
                                         bias=bias_sb[:, 0:1])
                    nc.vector.tensor_tensor(out=r_sb[:, :w], in0=gi[0][:, :w],
                                            in1=r_sb[:, :w], op=mybir.AluOpType.add)
                    nc.scalar.activation(out=r_sb[:, :w], in_=r_sb[:, :w],
                                         func=mybir.ActivationFunctionType.Sigmoid)
                    z_sb = sb.tile([P, W], _F32, tag="z")
                    nc.scalar.activation(out=z_sb[:, :w], in_=gh[1][:, :w],
                                         func=mybir.ActivationFunctionType.Identity,
                                         bias=bias_sb[:, 1:2])
                    nc.vector.tensor_tensor(out=z_sb[:, :w], in0=gi[1][:, :w],
                                            in1=z_sb[:, :w], op=mybir.AluOpType.add)
                    nc.scalar.activation(out=z_sb[:, :w], in_=z_sb[:, :w],
                                         func=mybir.ActivationFunctionType.Sigmoid)
                    hn_sb = sb.tile([P, W], _F32, tag="hn")
                    nc.scalar.activation(out=hn_sb[:, :w], in_=gh[2][:, :w],
                                         func=mybir.ActivationFunctionType.Identity,
                                         bias=bias_sb[:, 3:4])
                    nc.vector.tensor_tensor(out=hn_sb[:, :w], in0=r_sb[:, :w],
                                            in1=hn_sb[:, :w], op=mybir.AluOpType.mult)
                    nc.vector.tensor_tensor(out=hn_sb[:, :w], in0=hn_sb[:, :w],
                                            in1=gi[2][:, :w], op=mybir.AluOpType.add)
                    nc.scalar.activation(out=hn_sb[:, :w], in_=hn_sb[:, :w],
                                         func=mybir.ActivationFunctionType.Tanh,
                                         bias=bias_sb[:, 2:3])
                    d_sb = sb.tile([P, W], _F32, tag="d")
                    nc.vector.tensor_tensor(out=d_sb[:, :w], in0=hT[l % 2][:, cs],
                                            in1=hn_sb[:, :w],
                                            op=mybir.AluOpType.subtract)
                    nc.vector.tensor_tensor(out=d_sb[:, :w], in0=z_sb[:, :w],
                                            in1=d_sb[:, :w], op=mybir.AluOpType.mult)
                    nc.vector.tensor_tensor(out=hT_next[:, cs], in0=d_sb[:, :w],
                                            in1=hn_sb[:, :w], op=mybir.AluOpType.add)

            # ---- phase 1: embed gather straight into hnorm (layer-0 h) ----
            for r in range(NRANGE):
                if CAPR[r] == 0:
                    continue
                o = int(EmbOff[r])
                nc.gpsimd.dma_gather(
                    out_ap=hnorm[:, o * D:(o + CAPR[r]) * D].rearrange(
                        "p (t d) -> p t d", d=D),
                    in_ap=embed_in[r * VR:(r + 1) * VR, :],
                    idxs_ap=idxe_sb[:, o * 8:(o + CAPR[r]) * 8],
                    num_idxs=CAPR[r] * P,
                    num_idxs_reg=enregs[CAPR[r] * P],
                    elem_size=D, single_packet=False,
                    queue_num=qctr[0] % 4)
                qctr[0] += 1
            for b in range(NB):
                tp = pp.tile([P, P], _BF16, tag="scratch", space="PSUM")
                nc.tensor.transpose(out=tp[:], in_=hnorm[:, b * D:(b + 1) * D],
                                    identity=ident[:])
                nc.scalar.copy(out=hT[0][:, b * P:(b + 1) * P], in_=tp[:])

            # ---- layer 0: edge gathers straight from the embed table ----
            edge_layer(0, [embed_in[r * VR:(r + 1) * VR, :] for r in range(NRANGE)])
            gru_layer(0)

            # transpose h1 to row-major, ship to AllGather
            for b in range(NB):
                tp = pp.tile([P, P], _BF16, tag="scratch", space="PSUM")
                nc.tensor.transpose(out=tp[:], in_=hT[1][:, b * P:(b + 1) * P],
                                    identity=ident[:])
                nc.scalar.copy(out=hnorm[:, b * D:(b + 1) * D], in_=tp[:])
            nc.sync.dma_start(
                out=ag_in[:].rearrange("(b p) d -> p b d", p=P),
                in_=hnorm[:].rearrange("p (b d) -> p b d", d=D))
            nc.gpsimd.collective_compute(
                "AllGather", mybir.AluOpType.bypass,
                replica_groups=[list(range(NCORES))],
                ins=[ag_in[:]], outs=[ag_out[:]])

            # ---- layer 1: gathers from the all-gathered h ----
            edge_layer(1, [ag_out[0:HALF, :], ag_out[HALF:NFULL, :]])
            gru_layer(1)

            # ---- pool (h2 lives in hT[0]; transpose back to row-major) ----
            for b in range(NB):
                tp = pp.tile([P, P], _BF16, tag="scratch", space="PSUM")
                nc.tensor.transpose(out=tp[:], in_=hT[0][:, b * P:(b + 1) * P],
                                    identity=ident[:])
                nc.scalar.copy(out=hnorm[:, b * D:(b + 1) * D], in_=tp[:])
            ppool = pp.tile([G, D], _F32, tag="scratch", space="PSUM")
            for b in range(NB):
                nc.tensor.matmul(out=ppool[:], lhsT=pool_sb[:, b * G:(b + 1) * G],
                                 rhs=hnorm[:, b * D:(b + 1) * D],
                                 start=(b == 0), stop=(b == NB - 1))
            out_sb = sb.tile([G, D], _F32, tag="outsb")
            nc.vector.tensor_scalar(out=out_sb[:], in0=ppool[:],
                                    scalar1=cinv_sb[:, 0:1],
                                    scalar2=None, op0=mybir.AluOpType.mult)
            nc.sync.dma_start(out=out_ext[:], in_=out_sb[:])

    _split_waits(nc)
    lower_extended_insts(nc)
    return nc


def _wrap_idx(flat):
    """flat int16 [T*128] -> wrapped [128, T*8]: idx i at [g*16 + i%16, i//16]
    for g in 0..7."""
    cols = len(flat) // 16
    arr = flat.reshape(cols, 16).T            # [16, cols]
    return np.tile(arr, (8, 1)).astype(np.int16)


_CACHE = {}


def kernel(node_ids, edge_index, batch, num_graphs, embed, conv_w, w_ih, w_hh,
           b_ih, b_hh) -> np.ndarray:
    import ml_dtypes
    bf16 = ml_dtypes.bfloat16

    node_ids = np.asarray(node_ids).astype(np.int64)
    edge_index = np.asarray(edge_index).astype(np.int64)
    batch = np.asarray(batch).astype(np.int64)
    embed = np.asarray(embed, dtype=np.float32)
    conv_w = np.asarray(conv_w, dtype=np.float32)
    w_ih = np.asarray(w_ih, dtype=np.float32)
    w_hh = np.asarray(w_hh, dtype=np.float32)
    b_ih = np.asarray(b_ih, dtype=np.float32)
    b_hh = np.asarray(b_hh, dtype=np.float32)
    G_ = int(num_graphs)
    assert G_ == G and node_ids.shape[0] == N

    # ---- slot assignment: per core, nodes sorted by embed id, grouped into
    # NRANGE ranges, each range padded to a tile boundary ----
    ids_c = node_ids.reshape(NCORES, NL)
    orders = [np.argsort(ids_c[c], kind="stable") for c in range(NCORES)]
    nr_all = np.zeros((NCORES, NRANGE), np.int64)
    for c in range(NCORES):
        nr_all[c] = np.bincount(ids_c[c][orders[c]] // VR, minlength=NRANGE)
    CAPR = tuple(int(x) for x in np.ceil(nr_all.max(axis=0) / P).astype(int))
    NB = int(sum(CAPR))
    NLP = NB * P
    NFULL = NCORES * NLP
    HALF = NFULL // 2
    EmbOff = np.concatenate([[0], np.cumsum(CAPR)[:-1]]).astype(int)

    slot_of = np.empty(N, np.int64)          # global node -> local slot
    idxemb_flat = np.zeros((NCORES, sum(CAPR) * P), np.int16)
    for c in range(NCORES):
        o = orders[c]
        sids = ids_c[c][o]
        rng = sids // VR
        starts = np.searchsorted(rng, np.arange(NRANGE))
        ends = np.searchsorted(rng, np.arange(NRANGE), side="right")
        slot_sorted = np.empty(NL, np.int64)
        for r in range(NRANGE):
            n_r = ends[r] - starts[r]
            base = int(EmbOff[r]) * P
            slot_sorted[starts[r]:ends[r]] = base + np.arange(n_r)
            idxemb_flat[c, base:base + n_r] = (sids[starts[r]:ends[r]] - r * VR
                                               ).astype(np.int16)
        local = np.empty(NL, np.int64)
        local[o] = slot_sorted
        slot_of[c * NL:(c + 1) * NL] = local

    glob_slot = (np.arange(N) // NL) * NLP + slot_of

    # ---- edges: per layer, (group, idx16) of each edge's gather row ----
    src_all, dst_all = edge_index[0], edge_index[1]
    owner = dst_all // NL
    l0_gid = node_ids[src_all] // VR            # layer 0: embed table rows
    l0_idx = node_ids[src_all] % VR
    ps_all = glob_slot[src_all]                 # layer 1: padded global h rows
    l1_gid = (ps_all >= HALF).astype(np.int64)
    l1_idx = ps_all - l1_gid * HALF
    R = [NRANGE, 2]
    layer_gid = [l0_gid, l1_gid]
    layer_idx = [l0_idx, l1_idx]

    per_core = [[None] * NCORES for _ in range(NUM_LAYERS)]
    cnts = [np.zeros((NCORES, NB * R[l]), np.int64) for l in range(NUM_LAYERS)]
    for c in range(NCORES):
        sel = owner == c
        sd = slot_of[dst_all[sel]]
        blk_c = sd // P
        rel_c = sd % P
        for l in range(NUM_LAYERS):
            gid = layer_gid[l][sel]
            gidx = layer_idx[l][sel]
            key = blk_c * R[l] + gid
            oe = np.lexsort((gidx, key))
            ps, rel, kk = gidx[oe], rel_c[oe], key[oe]
            gs = np.ones(len(kk), bool)
            gs[1:] = kk[1:] != kk[:-1]
            new_slot = gs.copy()
            new_slot[1:] |= ps[1:] != ps[:-1]   # dedup same row within group
            cnt = np.bincount(kk[new_slot], minlength=NB * R[l])
            cnts[l][c] = cnt
            per_core[l][c] = (ps, rel, kk, new_slot, gs)

    caps = [np.ceil(cnts[l].max(axis=0) / P).astype(int).reshape(NB, R[l])
            for l in range(NUM_LAYERS)]
    SOff = [[np.concatenate([[0], np.cumsum(caps[l][:, r])[:-1]]).astype(int)
             for r in range(R[l])] for l in range(NUM_LAYERS)]
    TileOff = [np.concatenate([[0], np.cumsum(caps[l].sum(axis=1))[:-1]]
                              ).astype(int) for l in range(NUM_LAYERS)]
    NG = [[(int(caps[l][:, r].sum()) + GMAX - 1) // GMAX for r in range(R[l])]
          for l in range(NUM_LAYERS)]
    T_TOT = [int(caps[l].sum()) for l in range(NUM_LAYERS)]
    GrpOff = [np.concatenate([np.zeros((NB, 1), int),
                              np.cumsum(caps[l], axis=1)[:, :-1]], axis=1)
              for l in range(NUM_LAYERS)]

    # ---- common tensors ----
    embed_bf = embed.astype(bf16)
    convw_arr = np.concatenate([conv_w[i] for i in range(NUM_LAYERS)],
                               axis=1).astype(bf16)
    wihT = np.ascontiguousarray(w_ih.T).astype(bf16)
    whhT = np.ascontiguousarray(w_hh.T).astype(bf16)
    biases = np.zeros((P, 4), np.float32)
    biases[:, 0] = b_ih[0:D] + b_hh[0:D]
    biases[:, 1] = b_ih[D:2 * D] + b_hh[D:2 * D]
    biases[:, 2] = b_ih[2 * D:3 * D]
    biases[:, 3] = b_hh[2 * D:3 * D]
    counts_g = np.bincount(batch, minlength=G).astype(np.float32)
    cinv = (1.0 / np.maximum(counts_g, 1.0)).reshape(G, 1).astype(np.float32)
    ident = np.eye(P, dtype=np.float32).astype(bf16)

    in_maps = []
    for c in range(NCORES):
        im = {
            "embed": embed_bf,
            "idxemb": _wrap_idx(idxemb_flat[c]),
            "ident": ident,
            "cinv": cinv,
            "convw": convw_arr,
            "wihT": wihT,
            "whhT": whhT,
            "biases": biases,
        }
        for l in range(NUM_LAYERS):
            ps, rel, kk, new_slot, gs = per_core[l][c]
            blk = kk // R[l]
            gid = kk % R[l]
            slot_cum = np.cumsum(new_slot) - 1
            grp_first = np.zeros(NB * R[l], np.int64)
            grp_first[kk[gs]] = slot_cum[gs]
            pos = slot_cum - grp_first[kk]      # per-edge slot within group

            for r in range(R[l]):
                flat = np.zeros(NG[l][r] * GMAX * P, np.int16)
                m = (gid == r) & new_slot
                flat[SOff[l][r][blk[m]] * P + pos[m]] = ps[m].astype(np.int16)
                im[f"idx{l}_{r}"] = _wrap_idx(flat)

            mtile = TileOff[l][blk] + GrpOff[l][blk, gid] + pos // P
            masks_f = np.zeros((P, T_TOT[l] * P), np.float32)
            np.add.at(masks_f, (pos % P, mtile * P + rel), 1.0)
            im[f"masks{l}"] = masks_f.astype(bf16)

        b_c = batch[c * NL:(c + 1) * NL]
        p1h = np.zeros((NLP, G), np.float32)
        p1h[slot_of[c * NL:(c + 1) * NL], b_c] = 1.0
        pool1h = np.zeros((P, NB * G), np.float32)
        for b in range(NB):
            pool1h[:, b * G:(b + 1) * G] = p1h[b * P:(b + 1) * P, :]
        im["pool1h"] = pool1h.astype(bf16)
        in_maps.append(im)

    sig = (NB, CAPR, tuple(int(x) for x in caps[0].ravel()),
           tuple(int(x) for x in caps[1].ravel()))
    if sig not in _CACHE:
        _CACHE[sig] = _build(sig)
    nc = _CACHE[sig]

    trace = bool(int(os.environ.get("BASS_GNN_TRACE", "0")))
    if trace:
        _install_ntff_hook()
    res = run_bass_kernel_spmd(nc, in_maps, core_ids=list(range(NCORES)),
                               trace=trace)
    if trace:
        kernel.last_exec_time_ns = res.exec_time_ns
        kernel.last_results = res
    outs = [r["out"] for r in res.results]
    return np.sum(np.stack(outs, 0), axis=0, dtype=np.float32)


kernel.last_exec_time_ns = None
kernel.last_results = None


# revision 15
# speedup vs baseline: 1.9952x; 1.0668x over previous
"""GatedConv GNN message passing on 8 TRN2 NeuronCores — v7.

Bottleneck model (HW-measured): per-row gathers cost ~8.2ns/row of Q7
SWDGE descriptor generation per queue; 4 SWDGE queues run concurrently
(~2.4ns/row effective), so the gather pipeline dominates. Design:

- All row gathers use batched InstDMAGatherAnt (gpsimd.dma_gather, mlp
  ucode library), round-robin over 4 SWDGE queues, uniform 28-tile
  (3584-row) chunks sharing one num_idxs register.
- Layer 0 gathers per-edge h directly from the (bf16) embedding table —
  no AllGather needed for layer 0, and its gathers start at t=0. Only
  layer 1 needs an AllGather of the GRU output.
- dma_gather idxs are int16, so each gather series reads a <=32768-row
  table slice: 4 embed-id ranges for layer 0, lo/hi halves of the padded
  global node space for layer 1.
- Edges sharded by dst owner, grouped into 128-slot tiles per
  (dst-block, range-group); within a group, edges sharing a source row
  are DEDUPED into one gathered slot (the one-hot dst mask rows become
  multi-hot/counted). Per-(block,group) tile capacity uniform across
  cores.
- Aggregation: per tile, PE matmul (gathered.T @ mask) accumulated in
  PSUM per dst block = transposed segment sum. Conv weight folded after
  aggregation (linearity). Masks are host-built, stored [128, T*128] so
  each per-block load is one contiguous HWDGE DMA on the Activation
  queue.
- GRU runs in transposed [feature, node] layout; PE transposes produce
  row-major h only where needed (pooling, AllGather input).
- Mean-pool via host-built batch one-hot matmul + 1/count scale; host
  sums the 8 per-core partials.
"""
import contextlib
import os
import sys
import types

import numpy as np

from concourse import bass, mybir, tile, library_config
from concourse.bass_utils import run_bass_kernel_spmd
from concourse.library_overlay import lower_extended_insts

NCORES = 8
P = 128
D = 128
G = 64
N = 50000
V = 100000
NUM_LAYERS = 2
NL = N // NCORES            # 6250 nodes per core
NRANGE = 4                  # embed id ranges (V/NRANGE < 32768)
VR = V // NRANGE            # 25000
GMAX = 28                   # tiles per dma_gather (3584 idxs)

_F32 = mybir.dt.float32
_BF16 = mybir.dt.bfloat16
_I16 = mybir.dt.int16


def _split_waits(nc):
    """walrus allows only ONE sync-wait per instruction; hoist extras onto
    NoOps just before, on the same engine stream (sequencer order)."""
    uid = 0
    for bb in nc.main_func.blocks:
        out = []
        for ins in bb.instructions:
            si = getattr(ins, "sync_info", None)
            if si is not None and len(si.on_wait) > 1:
                for w in si.on_wait[:-1]:
                    uid += 1
                    out.append(mybir.InstNoOp(
                        name=f"WSPLIT-{uid}", engine=ins.engine,
                        bass_nofuse=True, ins=[], outs=[],
                        sync_info=mybir.SyncInfo(on_wait=[w], on_update=[]),
                    ))
                ins.sync_info = mybir.SyncInfo(
                    on_wait=[si.on_wait[-1]], on_update=si.on_update)
            out.append(ins)
        bb.instructions = out


def _install_ntff_hook():
    import antenv
    if "antenv.axon_hooks" in sys.modules:
        return
    mod = types.ModuleType("antenv.axon_hooks")
    _state = {"hook": None}
    mod.set_axon_ntff_profile_hook = lambda h: _state.__setitem__("hook", h)
    mod.get_axon_ntff_profile_hook = lambda: _state["hook"]
    sys.modules["antenv.axon_hooks"] = mod
    antenv.axon_hooks = mod
    if "/root/.axon_site" not in sys.path:
        sys.path.insert(0, "/root/.axon_site")
    try:
        from trn_agent_boot.trn_boot import _ntff_profile_via_ctypes
        hook = _ntff_profile_via_ctypes("/opt/axon/libaxon_pjrt.so")
        mod.set_axon_ntff_profile_hook(hook)
    except Exception:
        pass


# ---------------------------------------------------------------- builder
def _build(sig):
    """sig = (NB, CAPR, caps0, caps1); capsL = flattened [NB][R_l] tile caps."""
    NB, CAPR, caps0_f, caps1_f = sig
    NB = int(NB)
    CAPR = list(CAPR)
    R = [NRANGE, 2]
    caps = [np.array(caps0_f, int).reshape(NB, R[0]),
            np.array(caps1_f, int).reshape(NB, R[1])]
    NLP = NB * P
    NFULL = NCORES * NLP
    HALF = NFULL // 2
    T_EMB = sum(CAPR)
    EmbOff = np.concatenate([[0], np.cumsum(CAPR)[:-1]]).astype(int)

    SOff = []       # SOff[l][r][b] : tile offset of block b in series r
    TileOff = []    # TileOff[l][b] : mask-order tile offset of block b
    NG = []         # NG[l][r] : number of GMAX windows in series r
    CAPB = []       # CAPB[l][b] : total tiles of block b
    GrpOff = []     # GrpOff[l][b][r] : within-block tile offset of group r
    T_TOT = []
    for l in range(NUM_LAYERS):
        cl = caps[l]
        SOff.append([np.concatenate([[0], np.cumsum(cl[:, r])[:-1]]).astype(int)
                     for r in range(R[l])])
        capb = cl.sum(axis=1)
        CAPB.append(capb.astype(int))
        TileOff.append(np.concatenate([[0], np.cumsum(capb)[:-1]]).astype(int))
        NG.append([(int(cl[:, r].sum()) + GMAX - 1) // GMAX for r in range(R[l])])
        GrpOff.append(np.concatenate(
            [np.zeros((NB, 1), int), np.cumsum(cl, axis=1)[:, :-1]], axis=1))
        T_TOT.append(int(capb.sum()))
    CAPMAX = int(max(CAPB[0].max(), CAPB[1].max()))

    nc = bass.Bass(num_devices=NCORES, num_swdge_queues=4)

    embed_in = nc.declare_dram_parameter("embed", [V, D], _BF16, isOutput=False)
    idxe_in = nc.declare_dram_parameter("idxemb", [P, T_EMB * 8], _I16, isOutput=False)
    idx_in = [[nc.declare_dram_parameter(f"idx{l}_{r}", [P, NG[l][r] * GMAX * 8],
                                         _I16, isOutput=False)
               for r in range(R[l])] for l in range(NUM_LAYERS)]
    mask_in = [nc.declare_dram_parameter(f"masks{l}", [P, T_TOT[l] * P], _BF16,
                                         isOutput=False)
               for l in range(NUM_LAYERS)]
    ident_in = nc.declare_dram_parameter("ident", [P, P], _BF16, isOutput=False)
    pool_in = nc.declare_dram_parameter("pool1h", [P, NB * G], _BF16, isOutput=False)
    cinv_in = nc.declare_dram_parameter("cinv", [G, 1], _F32, isOutput=False)
    convw_in = nc.declare_dram_parameter("convw", [D, NUM_LAYERS * D], _BF16, isOutput=False)
    wih_in = nc.declare_dram_parameter("wihT", [D, 3 * D], _BF16, isOutput=False)
    whh_in = nc.declare_dram_parameter("whhT", [D, 3 * D], _BF16, isOutput=False)
    bias_in = nc.declare_dram_parameter("biases", [P, 4], _F32, isOutput=False)
    out_ext = nc.declare_dram_parameter("out", [G, D], _F32, isOutput=True)

    ag_in = nc.dram_tensor("ag_in", [NLP, D], _BF16)
    ag_out = nc.dram_tensor("ag_out", [NFULL, D], _BF16, addr_space="Shared")

    with tile.TileContext(nc) as tc:
        with contextlib.ExitStack() as stk:
            const = stk.enter_context(tc.tile_pool(name="const", bufs=1))
            sb = stk.enter_context(tc.tile_pool(name="sb", bufs=3))
            gpool = stk.enter_context(tc.tile_pool(name="gpool", bufs=2))
            mpool = stk.enter_context(tc.tile_pool(name="mpool", bufs=3))
            pp = stk.enter_context(tc.tile_pool(name="pp", bufs=2, space="PSUM"))
            gpsum = stk.enter_context(tc.tile_pool(name="gpsum", bufs=1, space="PSUM"))

            # ---- constants ----
            idxe_sb = const.tile([P, T_EMB * 8], _I16)
            nc.sync.dma_start(out=idxe_sb[:], in_=idxe_in[:])
            idx_sb = [[const.tile([P, NG[l][r] * GMAX * 8], _I16,
                                  name=f"idx{l}{r}", tag=f"idx{l}{r}")
                       for r in range(R[l])] for l in range(NUM_LAYERS)]
            for l in range(NUM_LAYERS):
                for r in range(R[l]):
                    nc.sync.dma_start(out=idx_sb[l][r][:], in_=idx_in[l][r][:])
            ident = const.tile([P, P], _BF16)
            nc.sync.dma_start(out=ident[:], in_=ident_in[:])
            pool_sb = const.tile([P, NB * G], _BF16)
            nc.sync.dma_start(out=pool_sb[:], in_=pool_in[:])
            cinv_sb = const.tile([G, 1], _F32)
            nc.sync.dma_start(out=cinv_sb[:], in_=cinv_in[:])
            bias_sb = const.tile([P, 4], _F32)
            nc.sync.dma_start(out=bias_sb[:], in_=bias_in[:])
            convw_sb = const.tile([D, NUM_LAYERS * D], _BF16)
            nc.sync.dma_start(out=convw_sb[:], in_=convw_in[:])
            wih_sb = const.tile([D, 3 * D], _BF16)
            nc.sync.dma_start(out=wih_sb[:], in_=wih_in[:])
            whh_sb = const.tile([D, 3 * D], _BF16)
            nc.sync.dma_start(out=whh_sb[:], in_=whh_in[:])

            # gpsimd carries only the mlp ucode library + dma_gathers
            nc.gpsimd.load_library(library_config.mlp)
            gnreg = nc.gpsimd.to_reg(GMAX * P)
            enregs = {}
            for r in range(NRANGE):
                if CAPR[r] and CAPR[r] * P not in enregs:
                    enregs[CAPR[r] * P] = nc.gpsimd.to_reg(CAPR[r] * P)

            hT = [const.tile([P, NLP], _BF16, name=f"hT{i}", tag=f"hT{i}")
                  for i in range(2)]
            hnorm = const.tile([P, NLP], _BF16)
            aggT = const.tile([P, NLP], _BF16)

            qctr = [0]

            def edge_layer(l, table_slices):
                """table_slices[r] = DRAM AP of the gather table for series r."""
                bufs = {}
                cl = caps[l]

                def issue(r, g):
                    # tags shared across layers (phases are sequential) to
                    # halve the static gpool footprint
                    buf = gpool.tile([P, GMAX * D], _BF16, tag=f"g{r}")
                    bufs[(r, g)] = buf
                    nc.gpsimd.dma_gather(
                        out_ap=buf[:].rearrange("p (t d) -> p t d", d=D),
                        in_ap=table_slices[r],
                        idxs_ap=idx_sb[l][r][:, g * GMAX * 8:(g + 1) * GMAX * 8],
                        num_idxs=GMAX * P, num_idxs_reg=gnreg,
                        elem_size=D, single_packet=False,
                        queue_num=qctr[0] % 4)
                    qctr[0] += 1

                def blk_need(b):
                    need = 0
                    for r in range(R[l]):
                        if cl[b][r]:
                            need = max(need,
                                       (int(SOff[l][r][b]) + cl[b][r] - 1) // GMAX)
                    return need

                n_g = max(NG[l])
                next_b = 0
                for g in range(n_g):
                    for r in range(R[l]):
                        if g < NG[l][r]:
                            issue(r, g)
                    while next_b < NB and blk_need(next_b) <= g:
                        b = next_b
                        next_b += 1
                        capb = int(CAPB[l][b])
                        if capb == 0:
                            nc.vector.memset(aggT[:, b * P:(b + 1) * P], 0.0)
                            continue
                        mask = mpool.tile([P, CAPMAX * P], _BF16, tag="mask")
                        to = int(TileOff[l][b])
                        meng = nc.sync if b % 2 == 0 else nc.scalar
                        meng.dma_start(
                            out=mask[:, :capb * P],
                            in_=mask_in[l][:, to * P:(to + capb) * P])
                        pagg = pp.tile([P, P], _F32, tag="scratch", space="PSUM")
                        k = 0
                        for r in range(R[l]):
                            for t in range(cl[b][r]):
                                st = int(SOff[l][r][b]) + t
                                buf = bufs[(r, st // GMAX)]
                                col = st % GMAX
                                nc.tensor.matmul(
                                    out=pagg[:],
                                    lhsT=buf[:, col * D:(col + 1) * D],
                                    rhs=mask[:, k * P:(k + 1) * P],
                                    start=(k == 0), stop=(k == capb - 1))
                                k += 1
                        nc.vector.tensor_copy(out=aggT[:, b * P:(b + 1) * P],
                                              in_=pagg[:])
                assert next_b == NB

            def gru_layer(l):
                W = 512
                nslab = (NLP + W - 1) // W
                hT_next = hT[(l + 1) % 2]
                for s in range(nslab):
                    c0 = s * W
                    w = min(W, NLP - c0)
                    cs = slice(c0, c0 + w)
                    xt_ps = gpsum.tile([P, W], _F32, tag="gi0", space="PSUM")
                    nc.tensor.matmul(out=xt_ps[:, :w],
                                     lhsT=convw_sb[:, l * D:(l + 1) * D],
                                     rhs=aggT[:, cs], start=True, stop=True)
                    xt_sb = sb.tile([P, W], _BF16, tag="xtsb")
                    nc.scalar.copy(out=xt_sb[:, :w], in_=xt_ps[:, :w])

                    gi = []
                    gh = []
                    for gidx in range(3):
                        gps = gpsum.tile([P, W], _F32, tag=f"gi{gidx}", space="PSUM")
                        nc.tensor.matmul(out=gps[:, :w],
                                         lhsT=wih_sb[:, gidx * D:(gidx + 1) * D],
                                         rhs=xt_sb[:, :w], start=True, stop=True)
                        gi.append(gps)
                        hps = gpsum.tile([P, W], _F32, tag=f"gh{gidx}", space="PSUM")
                        nc.tensor.matmul(out=hps[:, :w],
                                         lhsT=whh_sb[:, gidx * D:(gidx + 1) * D],
                                         rhs=hT[l % 2][:, cs], start=True, stop=True)
                        gh.append(hps)

                    r_sb = sb.tile([P, W], _F32, tag="r")
                    nc.scalar.activation(out=r_sb[:, :w], in_=gh[0][:, :w],
                                         func=mybir.ActivationFunctionType.Identity,
                                         bias=bias_sb[:, 0:1])
                    nc.vector.tensor_tensor(out=r_sb[:, :w], in0=gi[0][:, :w],
                                            in1=r_sb[:, :w], op=mybir.AluOpType.add)
                    nc.scalar.activation(out=r_sb[:, :w], in_=r_sb[:, :w],
                                         func=mybir.ActivationFunctionType.Sigmoid)
                    z_sb = sb.tile([P, W], _F32, tag="z")
                    nc.scalar.activation(out=z_sb[:, :w], in_=gh[1][:, :w],
                                         func=mybir.ActivationFunctionType.Identity,
                                         bias=bias_sb[:, 1:2])
                    nc.vector.tensor_tensor(out=z_sb[:, :w], in0=gi[1][:, :w],
                                            in1=z_sb[:, :w], op=mybir.AluOpType.add)
                    nc.scalar.activation(out=z_sb[:, :w], in_=z_sb[:, :w],
                                         func=mybir.ActivationFunctionType.Sigmoid)
                    hn_sb = sb.tile([P, W], _F32, tag="hn")
                    nc.scalar.activation(out=hn_sb[:, :w], in_=gh[2][:, :w],
                                         func=mybir.ActivationFunctionType.Identity,
                                         bias=bias_sb[:, 3:4])
                    nc.vector.tensor_tensor(out=hn_sb[:, :w], in0=r_sb[:, :w],
                                            in1=hn_sb[:, :w], op=mybir.AluOpType.mult)
                    nc.vector.tensor_tensor(out=hn_sb[:, :w], in0=hn_sb[:, :w],
                                            in1=gi[2][:, :w], op=mybir.AluOpType.add)
                    nc.scalar.activation(out=hn_sb[:, :w], in_=hn_sb[:, :w],
                                         func=mybir.ActivationFunctionType.Tanh,
                                         bias=bias_sb[:, 2:3])
                    d_sb = sb.tile([P, W], _F32, tag="d")
                    nc.vector.tensor_tensor(out=d_sb[:, :w], in0=hT[l % 2][:, cs],
                                            in1=hn_sb[:, :w],
                                            op=mybir.AluOpType.subtract)
                    nc.vector.tensor_tensor(out=d_sb[:, :w], in0=z_sb[:, :w],
                                            in1=d_sb[:, :w], op=mybir.AluOpType.mult)
                    nc.vector.tensor_tensor(out=hT_next[:, cs], in0=d_sb[:, :w],
                                            in1=hn_sb[:, :w], op=mybir.AluOpType.add)

            # ---- phase 1: embed gather straight into hnorm (layer-0 h) ----
            for r in range(NRANGE):
                if CAPR[r] == 0:
                    continue
                o = int(EmbOff[r])
                nc.gpsimd.dma_gather(
                    out_ap=hnorm[:, o * D:(o + CAPR[r]) * D].rearrange(
                        "p (t d) -> p t d", d=D),
                    in_ap=embed_in[r * VR:(r + 1) * VR, :],
                    idxs_ap=idxe_sb[:, o * 8:(o + CAPR[r]) * 8],
                    num_idxs=CAPR[r] * P,
                    num_idxs_reg=enregs[CAPR[r] * P],
                    elem_size=D, single_packet=False,
                    queue_num=qctr[0] % 4)
                qctr[0] += 1
            for b in range(NB):
                tp = pp.tile([P, P], _BF16, tag="scratch", space="PSUM")
                nc.tensor.transpose(out=tp[:], in_=hnorm[:, b * D:(b + 1) * D],
                                    identity=ident[:])
                nc.scalar.copy(out=hT[0][:, b * P:(b + 1) * P], in_=tp[:])

            # ---- layer 0: edge gathers straight from the embed table ----
            edge_layer(0, [embed_in[r * VR:(r + 1) * VR, :] for r in range(NRANGE)])
            gru_layer(0)

            # transpose h1 to row-major, ship to AllGather
            for b in range(NB):
                tp = pp.tile([P, P], _BF16, tag="scratch", space="PSUM")
                nc.tensor.transpose(out=tp[:], in_=hT[1][:, b * P:(b + 1) * P],
                                    identity=ident[:])
                nc.scalar.copy(out=hnorm[:, b * D:(b + 1) * D], in_=tp[:])
            nc.sync.dma_start(
                out=ag_in[:].rearrange("(b p) d -> p b d", p=P),
                in_=hnorm[:].rearrange("p (b d) -> p b d", d=D))
            nc.gpsimd.collective_compute(
                "AllGather", mybir.AluOpType.bypass,
                replica_groups=[list(range(NCORES))],
                ins=[ag_in[:]], outs=[ag_out[:]])

            # ---- layer 1: gathers from the all-gathered h ----
            edge_layer(1, [ag_out[0:HALF, :], ag_out[HALF:NFULL, :]])
            gru_layer(1)

            # ---- pool (h2 lives in hT[0]; transpose back to row-major) ----
            for b in range(NB):
                tp = pp.tile([P, P], _BF16, tag="scratch", space="PSUM")
                nc.tensor.transpose(out=tp[:], in_=hT[0][:, b * P:(b + 1) * P],
                                    identity=ident[:])
                nc.scalar.copy(out=hnorm[:, b * D:(b + 1) * D], in_=tp[:])
            ppool = pp.tile([G, D], _F32, tag="scratch", space="PSUM")
            for b in range(NB):
                nc.tensor.matmul(out=ppool[:], lhsT=pool_sb[:, b * G:(b + 1) * G],
                                 rhs=hnorm[:, b * D:(b + 1) * D],
                                 start=(b == 0), stop=(b == NB - 1))
            out_sb = sb.tile([G, D], _F32, tag="outsb")
            nc.vector.tensor_scalar(out=out_sb[:], in0=ppool[:],
                                    scalar1=cinv_sb[:, 0:1],
                                    scalar2=None, op0=mybir.AluOpType.mult)
            nc.sync.dma_start(out=out_ext[:], in_=out_sb[:])

    _split_waits(nc)
    lower_extended_insts(nc)
    return nc


def _wrap_idx(flat):
    """flat int16 [T*128] -> wrapped [128, T*8]: idx i at [g*16 + i%16, i//16]
    for g in 0..7."""
    cols = len(flat) // 16
    arr = flat.reshape(cols, 16).T            # [16, cols]
    return np.tile(arr, (8, 1)).astype(np.int16)


_CACHE = {}


def kernel(node_ids, edge_index, batch, num_graphs, embed, conv_w, w_ih, w_hh,
           b_ih, b_hh) -> np.ndarray:
    import ml_dtypes
    bf16 = ml_dtypes.bfloat16

    node_ids = np.asarray(node_ids).astype(np.int64)
    edge_index = np.asarray(edge_index).astype(np.int64)
    batch = np.asarray(batch).astype(np.int64)
    embed = np.asarray(embed, dtype=np.float32)
    conv_w = np.asarray(conv_w, dtype=np.float32)
    w_ih = np.asarray(w_ih, dtype=np.float32)
    w_hh = np.asarray(w_hh, dtype=np.float32)
    b_ih = np.asarray(b_ih, dtype=np.float32)
    b_hh = np.asarray(b_hh, dtype=np.float32)
    G_ = int(num_graphs)
    assert G_ == G and node_ids.shape[0] == N

    # ---- slot assignment: per core, nodes sorted by embed id, grouped into
    # NRANGE ranges, each range padded to a tile boundary ----
    ids_c = node_ids.reshape(NCORES, NL)
    orders = [np.argsort(ids_c[c], kind="stable") for c in range(NCORES)]
    nr_all = np.zeros((NCORES, NRANGE), np.int64)
    for c in range(NCORES):
        nr_all[c] = np.bincount(ids_c[c][orders[c]] // VR, minlength=NRANGE)
    CAPR = tuple(int(x) for x in np.ceil(nr_all.max(axis=0) / P).astype(int))
    NB = int(sum(CAPR))
    NLP = NB * P
    NFULL = NCORES * NLP
    HALF = NFULL // 2
    EmbOff = np.concatenate([[0], np.cumsum(CAPR)[:-1]]).astype(int)

    slot_of = np.empty(N, np.int64)          # global node -> local slot
    idxemb_flat = np.zeros((NCORES, sum(CAPR) * P), np.int16)
    for c in range(NCORES):
        o = orders[c]
        sids = ids_c[c][o]
        rng = sids // VR
        starts = np.searchsorted(rng, np.arange(NRANGE))
        ends = np.searchsorted(rng, np.arange(NRANGE), side="right")
        slot_sorted = np.empty(NL, np.int64)
        for r in range(NRANGE):
            n_r = ends[r] - starts[r]
            base = int(EmbOff[r]) * P
            slot_sorted[starts[r]:ends[r]] = base + np.arange(n_r)
            idxemb_flat[c, base:base + n_r] = (sids[starts[r]:ends[r]] - r * VR
                                               ).astype(np.int16)
        local = np.empty(NL, np.int64)
        local[o] = slot_sorted
        slot_of[c * NL:(c + 1) * NL] = local

    glob_slot = (np.arange(N) // NL) * NLP + slot_of

    # ---- edges: per layer, (group, idx16) of each edge's gather row ----
    src_all, dst_all = edge_index[0], edge_index[1]
    owner = dst_all // NL
    l0_gid = node_ids[src_all] // VR            # layer 0: embed table rows
    l0_idx = node_ids[src_all] % VR
    ps_all = glob_slot[src_all]                 # layer 1: padded global h rows
    l1_gid = (ps_all >= HALF).astype(np.int64)
    l1_idx = ps_all - l1_gid * HALF
    R = [NRANGE, 2]
    layer_gid = [l0_gid, l1_gid]
    layer_idx = [l0_idx, l1_idx]

    per_core = [[None] * NCORES for _ in range(NUM_LAYERS)]
    cnts = [np.zeros((NCORES, NB * R[l]), np.int64) for l in range(NUM_LAYERS)]
    for c in range(NCORES):
        sel = owner == c
        sd = slot_of[dst_all[sel]]
        blk_c = sd // P
        rel_c = sd % P
        for l in range(NUM_LAYERS):
            gid = layer_gid[l][sel]
            gidx = layer_idx[l][sel]
            key = blk_c * R[l] + gid
            oe = np.lexsort((gidx, key))
            ps, rel, kk = gidx[oe], rel_c[oe], key[oe]
            gs = np.ones(len(kk), bool)
            gs[1:] = kk[1:] != kk[:-1]
            new_slot = gs.copy()
            new_slot[1:] |= ps[1:] != ps[:-1]   # dedup same row within group
            cnt = np.bincount(kk[new_slot], minlength=NB * R[l])
            cnts[l][c] = cnt
            per_core[l][c] = (ps, rel, kk, new_slot, gs)

    caps = [np.ceil(cnts[l].max(axis=0) / P).astype(int).reshape(NB, R[l])
            for l in range(NUM_LAYERS)]
    SOff = [[np.concatenate([[0], np.cumsum(caps[l][:, r])[:-1]]).astype(int)
             for r in range(R[l])] for l in range(NUM_LAYERS)]
    TileOff = [np.concatenate([[0], np.cumsum(caps[l].sum(axis=1))[:-1]]
                              ).astype(int) for l in range(NUM_LAYERS)]
    NG = [[(int(caps[l][:, r].sum()) + GMAX - 1) // GMAX for r in range(R[l])]
          for l in range(NUM_LAYERS)]
    T_TOT = [int(caps[l].sum()) for l in range(NUM_LAYERS)]
    GrpOff = [np.concatenate([np.zeros((NB, 1), int),
                              np.cumsum(caps[l], axis=1)[:, :-1]], axis=1)
              for l in range(NUM_LAYERS)]

    # ---- common tensors ----
    embed_bf = embed.astype(bf16)
    convw_arr = np.concatenate([conv_w[i] for i in range(NUM_LAYERS)],
                               axis=1).astype(bf16)
    wihT = np.ascontiguousarray(w_ih.T).astype(bf16)
    whhT = np.ascontiguousarray(w_hh.T).astype(bf16)
    biases = np.zeros((P, 4), np.float32)
    biases[:, 0] = b_ih[0:D] + b_hh[0:D]
    biases[:, 1] = b_ih[D:2 * D] + b_hh[D:2 * D]
    biases[:, 2] = b_ih[2 * D:3 * D]
    biases[:, 3] = b_hh[2 * D:3 * D]
    counts_g = np.bincount(batch, minlength=G).astype(np.float32)
    cinv = (1.0 / np.maximum(counts_g, 1.0)).reshape(G, 1).astype(np.float32)
    ident = np.eye(P, dtype=np.float32).astype(bf16)

    in_maps = []
    for c in range(NCORES):
        im = {
            "embed": embed_bf,
            "idxemb": _wrap_idx(idxemb_flat[c]),
            "ident": ident,
            "cinv": cinv,
            "convw": convw_arr,
            "wihT": wihT,
            "whhT": whhT,
            "biases": biases,
        }
        for l in range(NUM_LAYERS):
            ps, rel, kk, new_slot, gs = per_core[l][c]
            blk = kk // R[l]
            gid = kk % R[l]
            slot_cum = np.cumsum(new_slot) - 1
            grp_first = np.zeros(NB * R[l], np.int64)
            grp_first[kk[gs]] = slot_cum[gs]
            pos = slot_cum - grp_first[kk]      # per-edge slot within group

            for r in range(R[l]):
                flat = np.zeros(NG[l][r] * GMAX * P, np.int16)
                m = (gid == r) & new_slot
                flat[SOff[l][r][blk[m]] * P + pos[m]] = ps[m].astype(np.int16)
                im[f"idx{l}_{r}"] = _wrap_idx(flat)

            mtile = TileOff[l][blk] + GrpOff[l][blk, gid] + pos // P
            masks_f = np.zeros((P, T_TOT[l] * P), np.float32)
            np.add.at(masks_f, (pos % P, mtile * P + rel), 1.0)
            im[f"masks{l}"] = masks_f.astype(bf16)

        b_c = batch[c * NL:(c + 1) * NL]
        p1h = np.zeros((NLP, G), np.float32)
        p1h[slot_of[c * NL:(c + 1) * NL], b_c] = 1.0
        pool1h = np.zeros((P, NB * G), np.float32)
        for b in range(NB):
            pool1h[:, b * G:(b + 1) * G] = p1h[b * P:(b + 1) * P, :]
        im["pool1h"] = pool1h.astype(bf16)
        in_maps.append(im)

    sig = (NB, CAPR, tuple(int(x) for x in caps[0].ravel()),
           tuple(int(x) for x in caps[1].ravel()))
    if sig not in _CACHE:
        _CACHE[sig] = _build(sig)
    nc = _CACHE[sig]

    trace = bool(int(os.environ.get("BASS_GNN_TRACE", "0")))
    if trace:
        _install_ntff_hook()
    res = run_bass_kernel_spmd(nc, in_maps, core_ids=list(range(NCORES)),
                               trace=trace)
    if trace:
        kernel.last_exec_time_ns = res.exec_time_ns
        kernel.last_results = res
    outs = [r["out"] for r in res.results]
    return np.sum(np.stack(outs, 0), axis=0, dtype=np.float32)


kernel.last_exec_time_ns = None
kernel.last_results = None
